# revision 24
# baseline (speedup 1.0000x reference)
"""Trainium2 Bass kernel: 3x3 VALID conv (NHWC, 256->256 ch) with weight
thresholding + bias, batch-sharded across 8 NeuronCores (4 images/core).

Algorithm: vertical 1D Winograd F(4,3) over kh — halves tensor-engine work
vs direct conv. All fp16 operands, fp32 PSUM accumulation (max rel err
~5e-3 vs the 2e-2 gate).

Engine split per core:
  - DVE: 8 fused scalar_tensor_tensor input-transform ops per (img, ct)
    + most of the inverse transform.
  - Pool (gpsimd): the plain tensor_tensor ops (4 of the input transform,
    u/v/t1/t2 of the inverse) — keeps DVE, the bottleneck, lighter.
  - PE: per (chunk of 8 vtiles, co, p): 6 accumulating fp16 matmuls
    (2 cin-tiles x 3 kw shifts) of 496 rows; FWL weight loads hidden.
  - ACT: PSUM -> fp16 M drains, plain Copy only (Identity+bias is 1.8x
    slower, so bias is folded into inverse stt ops instead).
"""

import sys

sys.path.insert(0, "/opt/trn_rl_repo")

import numpy as np

import concourse.bacc as bacc
import concourse.mybir as mybir
import concourse.tile as tile
from concourse.bass_utils import run_bass_kernel_spmd

F32 = mybir.dt.float32
F16 = mybir.dt.float16
ADD = mybir.AluOpType.add
SUB = mybir.AluOpType.subtract
MULT = mybir.AluOpType.mult
COPY = mybir.ActivationFunctionType.Copy
IDENT = mybir.ActivationFunctionType.Identity

N_CORES = 8
IMG_PER_CORE = 4
C = 256
NV = 62 * 62              # 3844 valid output pixels per image
NXROW = 66                # 64 input rows + 2 zero pad rows
SPARSE_TH = 0.01

G_MAT = np.array([
    [1 / 4, 0, 0],
    [-1 / 6, -1 / 6, -1 / 6],
    [-1 / 6, 1 / 6, -1 / 6],
    [1 / 24, 1 / 12, 1 / 6],
    [1 / 24, -1 / 12, 1 / 6],
    [0, 0, 1],
], dtype=np.float64)

_CACHE = {}

# Ablation knobs (timing experiments only; default = full kernel)
SKIP_TF = False
SKIP_INV = False
SKIP_MM = False


def _tf_ct(nc, x_d, xp, vp, tp, img, ct):
    """DMA x(img, ct) and emit its input transform. Returns the V tile."""
    stt = nc.vector.scalar_tensor_tensor
    ttp = nc.gpsimd.tensor_tensor

    xt = xp.tile([128, NXROW, 64], F16, tag=f"x{ct}", name="xt")
    nc.sync.dma_start(xt[:], x_d[img, ct])

    vt = vp.tile([128, 6, 16, 64], F16, tag=f"v{ct}", name="vt")
    t1 = tp.tile([128, 16, 64], F16, tag=f"t1{ct}", name="t1")
    t2 = tp.tile([128, 16, 64], F16, tag=f"t2{ct}", name="t2")

    def d(r):
        return xt[:, r:r + 61:4, :]

    V = lambda p: vt[:, p]
    if SKIP_TF:
        nc.vector.memset(vt[:], 0.0)
        return vt
    # V0 = 4 d0 - 5 d2 + d4 ; V5 = 4 d1 - 5 d3 + d5   (DVE)
    stt(t1[:], d(2), -5.0, d(4), MULT, ADD)
    stt(V(0), d(0), 4.0, t1[:], MULT, ADD)
    stt(t2[:], d(3), -5.0, d(5), MULT, ADD)
    stt(V(5), d(1), 4.0, t2[:], MULT, ADD)
    # s = d4 - 4 d2 ; p = 4 d1 - d3 (DVE) ; V1/V2 = s -/+ p (Pool)
    stt(t1[:], d(2), -4.0, d(4), MULT, ADD)
    stt(t2[:], d(1), 4.0, d(3), MULT, SUB)
    ttp(V(1), t1[:], t2[:], SUB)
    ttp(V(2), t1[:], t2[:], ADD)
    # u = d1 - d3 ; v = d4 - d2 (Pool) ; V3/V4 = -/+2u + v (DVE)
    ttp(t1[:], d(1), d(3), SUB)
    ttp(t2[:], d(4), d(2), SUB)
    stt(V(3), t1[:], -2.0, t2[:], MULT, ADD)
    stt(V(4), t1[:], 2.0, t2[:], MULT, ADD)
    return vt


def _tf_stage(nc, x_d, xp, vp, tp, img):
    return [_tf_ct(nc, x_d, xp, vp, tp, img, ct) for ct in range(2)]


def _emit_rep(nc, x_d, o_d, w_sb, b_sb, xp, vp, tp, pp, mp, op):
    stt = nc.vector.scalar_tensor_tensor
    ttv = nc.vector.tensor_tensor
    ttp = nc.gpsimd.tensor_tensor
    tsa = nc.vector.tensor_scalar_add

    v_cur = _tf_stage(nc, x_d, xp, vp, tp, 0)
    for img in range(IMG_PER_CORE):
        v_sb = v_cur
        v_cur = [None, None]

        # ---- matmuls + drains + inverse transform ----
        # Software pipelining: emit next image's transform (DVE/Pool)
        # before this image's inverse ops, so the vector engines run
        # ahead while the PE streams this image's matmuls.
        m_co = [mp.tile([128, 6, 992], F16, tag=f"m{co}", name="m")
                for co in range(2)]
        for chunk in range(2):
            vt0 = chunk * 8
            for co in range(2):
                if img + 1 < IMG_PER_CORE and chunk == 0:
                    # interleave next image's per-ct transform with this
                    # image's matmul groups
                    v_cur[co] = _tf_ct(nc, x_d, xp, vp, tp, img + 1, co)
                m = m_co[co]
                for p in range(6):
                    if SKIP_MM:
                        if p == 0:
                            nc.vector.memset(m[:], 0.0)
                        continue
                    ps = pp.tile([128, 496], F32, tag="ps", name="ps")
                    k = 0
                    for ct in range(2):
                        for kw in range(3):
                            off = ct * 4608 + p * 768 + kw * 256 + co * 128
                            nc.tensor.matmul(
                                ps[:],
                                w_sb[:, off:off + 128],
                                v_sb[ct][:, p, vt0:vt0 + 8, kw:kw + 62],
                                start=(k == 0), stop=(k == 5),
                            )
                            k += 1
                    nc.scalar.activation(
                        m[:, p, chunk * 496:(chunk + 1) * 496], ps[:],
                        COPY)

        # inverse, both chunks at once (N=992): out[4t+i] = AT[i].M + bias
        for co in range(2):
            m = m_co[co]
            ob = op.tile([128, 16, 4, 62], F16, tag="ob", name="ob")
            if SKIP_INV:
                nc.vector.tensor_copy(ob[:], m[:, 0:4])
            else:
                i1 = tp.tile([128, 992], F16, tag="i1", name="i1")
                i2 = tp.tile([128, 992], F16, tag="i2", name="i2")
                i3 = tp.tile([128, 992], F16, tag="i3", name="i3")
                M = lambda p: m[:, p]
                O = lambda i: ob[:, :, i, :]
                b = b_sb[:, co:co + 1]
                # t1 = M1+M2 ; t2 = M3+M4   (DVE)
                ttv(i1[:], M(1), M(2), ADD)
                ttv(i2[:], M(3), M(4), ADD)
                # out0 = ((M0 + b) + t1) + t2
                stt(i3[:], M(0), b, i1[:], ADD, ADD)
                ttv(O(0), i3[:], i2[:], ADD)
                # out2 = 4*t2 + t1 + b
                stt(i3[:], i2[:], 4.0, i1[:], MULT, ADD)
                tsa(O(2), i3[:], b)
                # u = M1-M2 ; v = M3-M4   (DVE)
                ttv(i1[:], M(1), M(2), SUB)
                ttv(i2[:], M(3), M(4), SUB)
                # out1 = 2v + u + b
                stt(i3[:], i2[:], 2.0, i1[:], MULT, ADD)
                tsa(O(1), i3[:], b)
                # out3 = (8v + u) + b + M5
                stt(i3[:], i2[:], 8.0, i1[:], MULT, ADD)
                stt(O(3), i3[:], b, M(5), ADD, ADD)

            nc.sync.dma_start(o_d[img, co, :, 0:3720], ob[:, 0:15])
            nc.sync.dma_start(o_d[img, co, :, 3720:3844], ob[:, 15, 0:2])


def _build(reps: int = 1, hw_loop: bool = False, internal_io: bool = False,
           unroll: int = 1):
    key = (reps, hw_loop, internal_io, unroll)
    if key in _CACHE:
        return _CACHE[key]

    nc = bacc.Bacc("TRN2", target_bir_lowering=False, debug=False,
                   num_devices=N_CORES)

    io_kind = "Internal" if internal_io else None
    x_d = nc.dram_tensor("xt", [IMG_PER_CORE, 2, 128, NXROW * 64], F16,
                         kind=io_kind or "ExternalInput")
    w_d = nc.dram_tensor("wt", [128, 2 * 4608], F16,
                         kind="ExternalInput")
    b_d = nc.dram_tensor("bias", [128, 2], F32, kind="ExternalInput")
    o_d = nc.dram_tensor("out", [IMG_PER_CORE, 2, 128, NV], F16,
                         kind=io_kind or "ExternalOutput")
    t_d = None
    if internal_io:
        t_d = nc.dram_tensor("tick", [128, 2], F32, kind="ExternalOutput")

    with tile.TileContext(nc) as tc:
        with tc.tile_pool(name="wp", bufs=1) as wp, \
             tc.tile_pool(name="xp", bufs=2) as xp, \
             tc.tile_pool(name="vp", bufs=2) as vp, \
             tc.tile_pool(name="tp", bufs=2) as tp, \
             tc.tile_pool(name="pp", bufs=8, space="PSUM") as pp, \
             tc.tile_pool(name="mp", bufs=2) as mp, \
             tc.tile_pool(name="op", bufs=3) as op:

            w_sb = wp.tile([128, 2 * 4608], F16, tag="w")
            nc.sync.dma_start(w_sb[:], w_d[:])
            b_sb = wp.tile([128, 2], F32, tag="bias")
            nc.sync.dma_start(b_sb[:], b_d[:])

            def rep():
                _emit_rep(nc, x_d, o_d, w_sb, b_sb,
                          xp, vp, tp, pp, mp, op)

            if hw_loop and reps > 1:
                assert reps % unroll == 0
                with tc.For_i(0, reps // unroll, 1):
                    for _ in range(unroll):
                        rep()
            else:
                for _ in range(reps):
                    rep()

            if t_d is not None:
                nc.sync.dma_start(t_d[:], b_sb[:])

    nc.compile()
    _CACHE[key] = nc
    return nc


def _prep_inputs(x, weight, bias):
    """Host-side shard prep: threshold mask, G-transform of weights,
    transpose+pad of x. Per-core in_maps."""
    w = np.where(np.abs(weight) < SPARSE_TH, 0.0, weight).astype(np.float64)
    wt = np.einsum('pr,ocrk->pkco', G_MAT, w)
    wt = wt.reshape(6, 3, 2, 128, 2, 128)          # p kw ct ci co o
    wt = wt.transpose(3, 2, 0, 1, 4, 5)            # ci ct p kw co o
    wt = np.ascontiguousarray(wt.reshape(128, 2 * 4608)).astype(np.float16)

    b2 = np.ascontiguousarray(
        bias.astype(np.float32).reshape(2, 128).T)

    n_img = x.shape[0]
    xs = x.astype(np.float32).reshape(n_img, 4096, C).transpose(0, 2, 1)
    xp = np.zeros((n_img, C, NXROW, 64), np.float16)
    xp[:, :, :64, :] = xs.reshape(n_img, C, 64, 64)
    xp = xp.reshape(n_img, 2, 128, NXROW * 64)

    in_maps = []
    for c in range(N_CORES):
        in_maps.append({
            "xt": np.ascontiguousarray(
                xp[c * IMG_PER_CORE:(c + 1) * IMG_PER_CORE]),
            "wt": wt,
            "bias": b2,
        })
    return in_maps


def _assemble(results):
    outs = np.concatenate([r["out"] for r in results], axis=0)
    outs = outs.astype(np.float32).reshape(32, C, 62, 62).transpose(0, 2, 3, 1)
    return np.ascontiguousarray(outs)


def kernel(x, weight, bias):
    x = np.asarray(x)
    weight = np.asarray(weight)
    bias = np.asarray(bias)
    nc = _build(reps=1)
    in_maps = _prep_inputs(x, weight, bias)
    res = run_bass_kernel_spmd(nc, in_maps, list(range(N_CORES)))
    return _assemble(res.results)
